# revision 1
# baseline (speedup 1.0000x reference)
"""Eq2to2 equivariant layer (Maron et al. 2-to-2 basis, 15 ops) as a Trainium2
Bass/Tile kernel, data-parallel over the batch axis N across 8 NeuronCores.

Math: the 15-basis contraction collapses to
  out[n,s] = sum_d C9[d,s]*x[n,d] + sum_d C10[d,s]*x[n,d]^T
           + Row[n,s,i] (bcast over j) + Col[n,s,j] (bcast over i)
           + delta_ij * DiagT[n,s,i] + Const[n,s] + bias[s] + delta_ij*diag_bias[s]
where Row/Col/DiagT/Const are small contractions of rowsum/colsum/diag/tot stats.

Layout: each core takes 4 n's -> 128 SBUF partitions = (nq, d). Grids are flat
in the free dim (16384 f32 per partition). The x^T einsum needs no data
movement: the matmul moving operand reads the grid through a transposed
strided AP ([[1,4],[128,128]]) within each partition.
"""

import sys

import numpy as np

if "/opt/trn_rl_repo" not in sys.path:
    sys.path.insert(0, "/opt/trn_rl_repo")

N, D, S, B, M = 32, 32, 32, 15, 128
NCORES = 8
NPC = N // NCORES          # n's per core = 4
P = 128                    # partitions
FREE = M * M               # 16384
CHUNK = 512                # psum bank (f32)
NCHUNK = FREE // CHUNK     # 32
OUTW = 2048                # out staging width (4 chunks)
NLOAD = 8                  # xa load slices
SL = FREE // NLOAD         # 2048 elements (16 i-rows) per load slice

_cache: dict = {}

# float32r: single-pass fp32 matmul (TF32-like mantissa on HW, 4x faster than
# exact fp32 for free-dim >= 256). Flip to False to fall back to exact fp32.
USE_F32R = True
ACT_CHUNKS = 16  # of the 32 chunks, how many get ACT-path assembly


def _build_program(repeat=1):
    import concourse.bass as bass
    import concourse.tile as tile
    from concourse import bacc, mybir

    f32 = mybir.dt.float32
    f32r = mybir.dt.float32r
    nc = bacc.Bacc("TRN2", target_bir_lowering=False, debug=False)

    xr_d = nc.dram_tensor("xr", [P, FREE], f32, kind="ExternalInput")
    # pre-scaled coefs [15, D, S]; blockdiag replication happens on-device
    wm_d = nc.dram_tensor("wmats", [15, D, S], f32, kind="ExternalInput")
    bc_d = nc.dram_tensor("bcols", [P, 2], f32, kind="ExternalInput")
    out_d = nc.dram_tensor("outr", [P, FREE], f32, kind="ExternalOutput")

    ADD = mybir.AluOpType.add
    IDENT = mybir.ActivationFunctionType.Identity

    with tile.TileContext(nc) as tc:
        with (
            tc.tile_pool(name="big", bufs=1) as big,
            tc.tile_pool(name="cst", bufs=1) as cst,
            tc.tile_pool(name="aux", bufs=1) as aux,
            tc.tile_pool(name="ot", bufs=3) as otp,
            tc.tile_pool(name="pm", bufs=6, space="PSUM") as pmp,
            tc.tile_pool(name="pa", bufs=1, space="PSUM") as pap,
        ):
          for _rep in range(repeat):
            # ---- constants ----
            # (loads bitcast to f32r so the verifier accepts f32r matmul use;
            #  host pre-rounds the data to fp32r precision)
            ldt = f32r if USE_F32R else f32
            wm = cst.tile([P, 15, P], f32)
            # build block-diagonal weight mats on-device: zero then drop the
            # [15, 32, 32] coef blocks onto the 4 diagonal positions
            nc.gpsimd.memset(wm[:], 0.0)
            for nq in range(NPC):
                nc.sync.dma_start(
                    out=wm[nq * D:(nq + 1) * D, :, nq * S:(nq + 1) * S].bitcast(ldt),
                    in_=wm_d[:].rearrange("w d s -> d w s").bitcast(ldt),
                )
            bc = cst.tile([P, 2], f32)
            nc.sync.dma_start(out=bc[:], in_=bc_d[:])

            W = lambda idx: wm[:, idx, :]
            (W_X, W_XT, W_ROW_CS, W_ROW_RS, W_ROW_DG, W_COL_CS, W_COL_RS,
             W_COL_DG, W_DIA_DG, W_DIA_RS, W_DIA_CS, W_SD_SD, W_SD_TOT,
             W_SC_SD, W_SC_TOT) = range(15)

            # ---- stats tiles ----
            rowsum = aux.tile([P, M], f32)   # rowsum[p, i] = sum_j x[p, i, j]
            colsum = aux.tile([P, M], f32)   # colsum[p, j] = sum_i x[p, i, j]
            diagx = aux.tile([P, M], f32)    # diag[p, i] = x[p, i, i]
            sd = aux.tile([P, 1], f32)       # sum of diag
            tot = aux.tile([P, 1], f32)      # total sum
            pacc = aux.tile([P, SL // 2], f32)   # colsum accumulator (Pool, slices 0-4)
            ptm2 = aux.tile([P, SL // 2], f32)   # per-slice pair sum (Pool)
            dacc = aux.tile([P, SL // 2], f32)   # colsum accumulator (DVE, slices 5-7)
            dtm2 = aux.tile([P, SL // 2], f32)   # per-slice pair sum (DVE)

            # ---- load x rows; stats per slice overlap the loads ----
            xa = big.tile([P, FREE], f32)
            xa_ap = xa[:]

            def ap(offset, dims):
                return bass.AP(
                    tensor=xa_ap.tensor,
                    offset=xa_ap.offset + offset,
                    ap=[list(xa_ap.ap[0])] + dims,
                )

            IPS = SL // M  # i-rows per slice = 16
            for t in range(NLOAD):
                sl = slice(t * SL, (t + 1) * SL)
                nc.sync.dma_start(out=xa[:, sl].bitcast(ldt),
                                  in_=xr_d[:, sl].bitcast(ldt))
                # rowsum of this slice's 16 i-rows (DVE)
                nc.vector.reduce_sum(
                    out=rowsum[:, t * IPS:(t + 1) * IPS],
                    in_=ap(t * SL, [[M, IPS], [1, M]]),
                    axis=mybir.AxisListType.X,
                )
                # colsum partials: fold each slice's 16 i-rows to 8 rows;
                # slices 0-5 chained on GPSIMD, 6-7 on DVE (late slices,
                # short tail after the last load lands)
                if t < 6:
                    eng, acc, tmp = nc.gpsimd, pacc, ptm2
                else:
                    eng, acc, tmp = nc.vector, dacc, dtm2
                dst = acc if t in (0, 6) else tmp
                eng.tensor_tensor(out=dst[:], in0=xa[:, t * SL: t * SL + SL // 2],
                                  in1=xa[:, t * SL + SL // 2:(t + 1) * SL], op=ADD)
                if t not in (0, 6):
                    eng.tensor_tensor(out=acc[:], in0=acc[:], in1=tmp[:], op=ADD)
            # merge accumulators + fold 8 i-rows into colsum (DVE, tiny)
            nc.vector.tensor_tensor(out=pacc[:], in0=pacc[:], in1=dacc[:], op=ADD)
            w = SL // 4
            while w > M:
                nc.vector.tensor_tensor(out=pacc[:, 0:w], in0=pacc[:, 0:w],
                                        in1=pacc[:, w:2 * w], op=ADD)
                w //= 2
            nc.vector.tensor_tensor(out=colsum[:], in0=pacc[:, 0:M],
                                    in1=pacc[:, M:2 * M], op=ADD)
            # diag: one strided copy (f = 129*i), then scalars
            nc.vector.tensor_copy(out=diagx[:], in_=ap(0, [[M + 1, M]]))
            nc.vector.reduce_sum(out=sd[:], in_=diagx[:], axis=mybir.AxisListType.X)
            nc.vector.reduce_sum(out=tot[:], in_=rowsum[:], axis=mybir.AxisListType.X)

            # ---- aux contractions over d (partition dim) on the PE ----
            pa = pap.tile([P, CHUNK], f32)  # sections: row | col | diag | scal
            mm = nc.tensor.matmul
            mm(pa[:, 0:M], W(W_ROW_CS), colsum[:], start=True, stop=False)
            mm(pa[:, 0:M], W(W_ROW_RS), rowsum[:], start=False, stop=False)
            mm(pa[:, 0:M], W(W_ROW_DG), diagx[:], start=False, stop=True)

            mm(pa[:, M:2 * M], W(W_COL_CS), colsum[:], start=True, stop=False)
            mm(pa[:, M:2 * M], W(W_COL_RS), rowsum[:], start=False, stop=False)
            mm(pa[:, M:2 * M], W(W_COL_DG), diagx[:], start=False, stop=True)

            mm(pa[:, 2 * M:3 * M], W(W_DIA_DG), diagx[:], start=True, stop=False)
            mm(pa[:, 2 * M:3 * M], W(W_DIA_RS), rowsum[:], start=False, stop=False)
            mm(pa[:, 2 * M:3 * M], W(W_DIA_CS), colsum[:], start=False, stop=True)

            mm(pa[:, 3 * M:3 * M + 1], W(W_SD_SD), sd[:], start=True, stop=False)
            mm(pa[:, 3 * M:3 * M + 1], W(W_SD_TOT), tot[:], start=False, stop=True)
            mm(pa[:, 3 * M + 1:3 * M + 2], W(W_SC_SD), sd[:], start=True, stop=False)
            mm(pa[:, 3 * M + 1:3 * M + 2], W(W_SC_TOT), tot[:], start=False, stop=True)

            # fold constants: RowF = Row + Const + bias; DiagF = DiagT + DiagConst + diag_bias
            rowf = aux.tile([P, M], f32)
            colf = aux.tile([P, M], f32)
            diaf = aux.tile([P, M], f32)
            nc.vector.tensor_scalar(out=rowf[:], in0=pa[:, 0:M],
                                    scalar1=pa[:, 3 * M + 1:3 * M + 2],
                                    scalar2=bc[:, 0:1], op0=ADD, op1=ADD)
            nc.scalar.copy(out=colf[:], in_=pa[:, M:2 * M])
            nc.vector.tensor_scalar(out=diaf[:], in0=pa[:, 2 * M:3 * M],
                                    scalar1=pa[:, 3 * M:3 * M + 1],
                                    scalar2=bc[:, 1:2], op0=ADD, op1=ADD)

            # ---- main einsum + assembly, streamed in 512-wide chunks ----
            cast = (lambda a: a.bitcast(f32r)) if USE_F32R else (lambda a: a)
            for g in range(NCHUNK // 4):  # output-staging groups of 4 chunks
                ot = otp.tile([P, OUTW], f32)
                for cc in range(4):
                    c = g * 4 + cc
                    i0 = 4 * c
                    pm = pmp.tile([P, CHUNK], f32, tag="pm")
                    # C9 term: contiguous grid chunk (rows i0..i0+3)
                    mm(pm[:], cast(W(W_X)), cast(xa[:, c * CHUNK:(c + 1) * CHUNK]),
                       start=True, stop=False)
                    # C10 term: transposed read of the same output window
                    mm(pm[:], cast(W(W_XT)), cast(ap(i0, [[1, 4], [M, M]])),
                       start=False, stop=True)
                    # out = (psum + RowF[i]) + ColF[j]
                    if (c % 2 == 0) and ACT_CHUNKS > 0:
                        # ACT path: psum + RowF via activation bias; ColF via
                        # one GPSIMD add with a broadcast (stride-0) AP
                        for q in range(4):
                            nc.scalar.activation(
                                out=ot[:, cc * CHUNK + q * M: cc * CHUNK + (q + 1) * M],
                                in_=pm[:, q * M:(q + 1) * M],
                                func=IDENT,
                                bias=rowf[:, i0 + q:i0 + q + 1],
                            )
                        cfb = bass.AP(tensor=colf[:].tensor, offset=colf[:].offset,
                                      ap=[list(colf[:].ap[0]), [0, 4], [1, M]])
                        otv = ot[:, cc * CHUNK:(cc + 1) * CHUNK].rearrange(
                            "p (i j) -> p i j", i=4)
                        nc.gpsimd.tensor_tensor(out=otv, in0=otv, in1=cfb, op=ADD)
                    else:
                        for q in range(4):
                            nc.vector.scalar_tensor_tensor(
                                out=ot[:, cc * CHUNK + q * M: cc * CHUNK + (q + 1) * M],
                                in0=pm[:, q * M:(q + 1) * M],
                                scalar=rowf[:, i0 + q:i0 + q + 1],
                                in1=colf[:],
                                op0=ADD, op1=ADD,
                            )
                    # diagonal add: positions f_local = cc*512 + i0 + 129*q
                    ot_ap = ot[:]
                    dview = bass.AP(
                        tensor=ot_ap.tensor,
                        offset=ot_ap.offset + cc * CHUNK + i0,
                        ap=[list(ot_ap.ap[0]), [M + 1, 4]],
                    )
                    nc.vector.tensor_tensor(out=dview, in0=dview,
                                            in1=diaf[:, i0:i0 + 4], op=ADD)
                nc.sync.dma_start(out=out_d[:, g * OUTW:(g + 1) * OUTW], in_=ot[:])

    nc.compile()
    return nc


def _get_nc():
    if "nc" not in _cache:
        _cache["nc"] = _build_program()
    return _cache["nc"]


def _host_prep(coefs, bias, diag_bias):
    m = float(M)
    C = np.asarray(coefs, dtype=np.float32)

    def bd(b, scale=1.0):
        return C[:, :, b] * np.float32(scale)

    # [15, D, S] pre-scaled coef blocks; blockdiag replication is on-device
    wmats = np.stack([
        bd(9),              # W_X
        bd(10),             # W_XT
        bd(5, 1 / m),       # W_ROW_CS
        bd(6, 1 / m),       # W_ROW_RS
        bd(11),             # W_ROW_DG
        bd(7, 1 / m),       # W_COL_CS
        bd(8, 1 / m),       # W_COL_RS
        bd(12),             # W_COL_DG
        bd(0),              # W_DIA_DG
        bd(2, 1 / m),       # W_DIA_RS
        bd(3, 1 / m),       # W_DIA_CS
        bd(1, 1 / m),       # W_SD_SD
        bd(4, 1 / (m * m)),  # W_SD_TOT
        bd(13, 1 / m),      # W_SC_SD
        bd(14, 1 / (m * m)),  # W_SC_TOT
    ]).astype(np.float32)
    bcols = np.stack([
        np.tile(np.asarray(bias, np.float32).reshape(S), NPC),
        np.tile(np.asarray(diag_bias, np.float32).reshape(S), NPC),
    ], axis=1).astype(np.float32)
    return np.ascontiguousarray(wmats), np.ascontiguousarray(bcols)


def _round_f32r(a):
    # fp32r-representable = exact sum of two bf16s (what the PE's single-pass
    # fp32 mode assumes); ~2^-16 relative rounding.
    import ml_dtypes

    hi = a.astype(ml_dtypes.bfloat16).astype(np.float32)
    lo = (a - hi).astype(ml_dtypes.bfloat16).astype(np.float32)
    return hi + lo


def _in_maps(inputs, coefs, bias, diag_bias):
    x = np.ascontiguousarray(np.asarray(inputs, np.float32))
    wmats, bcols = _host_prep(coefs, bias, diag_bias)
    if USE_F32R:
        x = _round_f32r(x)
        wmats = _round_f32r(wmats)
    maps = []
    for i in range(NCORES):
        xr = x[i * NPC:(i + 1) * NPC].reshape(P, FREE)
        maps.append({"xr": np.ascontiguousarray(xr), "wmats": wmats, "bcols": bcols})
    return maps


def run(inputs, coefs, bias, diag_bias, **spmd_kwargs):
    """Run on the 8 NeuronCores; returns (output, BassKernelResults)."""
    global USE_F32R
    from concourse.bass_utils import run_bass_kernel_spmd

    nc = _get_nc()
    maps = _in_maps(inputs, coefs, bias, diag_bias)
    try:
        res = run_bass_kernel_spmd(nc, maps, list(range(NCORES)), **spmd_kwargs)
    except Exception:
        if not USE_F32R:
            raise
        # fall back to exact fp32 matmuls if fp32r fails to compile/run here
        USE_F32R = False
        _cache.clear()
        nc = _get_nc()
        maps = _in_maps(inputs, coefs, bias, diag_bias)
        res = run_bass_kernel_spmd(nc, maps, list(range(NCORES)), **spmd_kwargs)
    out = np.concatenate(
        [r["outr"].reshape(NPC, S, M, M) for r in res.results], axis=0
    )
    return np.ascontiguousarray(out.astype(np.float32)), res


def kernel(inputs, coefs, bias, diag_bias):
    out, _ = run(inputs, coefs, bias, diag_bias)
    return out



# revision 17
# speedup vs baseline: 62.6430x; 62.6430x over previous
"""Eq2to2 equivariant layer (Maron et al. 2-to-2 basis, 15 ops) as a Trainium2
Bass/Tile kernel, data-parallel over the batch axis N across 8 NeuronCores.

Math: the 15-basis contraction collapses to
  out[n,s] = sum_d C9[d,s]*x[n,d] + sum_d C10[d,s]*x[n,d]^T
           + Row[n,s,i] (bcast over j) + Col[n,s,j] (bcast over i)
           + delta_ij * DiagT[n,s,i] + Const[n,s] + bias[s] + delta_ij*diag_bias[s]
where Row/Col/DiagT/Const are small contractions of rowsum/colsum/diag/tot stats.

This version is memory-roofline oriented: all HBM traffic is bf16 (host casts
both ways; tolerance 2e-2 >> bf16 rounding), halving DMA bytes vs f32.
Per 512-wide output chunk the PE runs THREE accumulating matmuls:
  psum  = W9^T  @ x_chunk                  (contiguous read)
  psum += W10^T @ x^T-chunk                (strided AP read, no data movement)
  psum += rowft[4c:4c+4]^T @ OnesBlk       (row-broadcast term, contract dim 4)
ACT drains psum->SBUF (1024-wide), DVE adds Col+Const+bias with one
broadcast-AP op per 4096-wide group and the sparse diagonal term, then the
group DMAs out. Row/col stats are computed with DVE pair-fold trees
(scalar_tensor_tensor runs in 4x DVE mode on packed bf16; TensorReduce has no
fast mode). Weights load compact [D,15,S] and scatter into a block-diagonal
[128,15,128] on device.

Layout: each core takes 4 n's -> 128 SBUF partitions = (nq, d). Grids are flat
in the free dim (16384 bf16 per partition).
"""

import sys

import numpy as np

if "/opt/trn_rl_repo" not in sys.path:
    sys.path.insert(0, "/opt/trn_rl_repo")

N, D, S, B, M = 32, 32, 32, 15, 128
NCORES = 8
NPC = N // NCORES          # n's per core = 4
P = 128                    # partitions
FREE = M * M               # 16384
CHUNK = 512                # psum half-bank pair (f32)
NCHUNK = FREE // CHUNK     # 32
NLOAD = 4                  # xa load slices (1 MiB bf16 each)
SL = FREE // NLOAD         # 4096 elements (32 i-rows) per load slice
IPS = SL // M              # i-rows per slice = 32
NGROUP = 4                 # out staging groups
GW = FREE // NGROUP        # 4096 elements per group (8 chunks)

_cache: dict = {}

ADD = None  # set in _build_program


def _build_program(repeat=1):
    import concourse.bass as bass
    import concourse.tile as tile
    from concourse import bacc, mybir

    f32 = mybir.dt.float32
    bf16 = mybir.dt.bfloat16
    nc = bacc.Bacc("TRN2", target_bir_lowering=False, debug=False)

    xr_d = nc.dram_tensor("xr", [P, FREE], bf16, kind="ExternalInput")
    # compact pre-scaled coefs [D, 15, S]; blockdiag scatter happens on-device
    wm_d = nc.dram_tensor("wmats", [D, 15, S], bf16, kind="ExternalInput")
    bc_d = nc.dram_tensor("bcols", [P, 2], f32, kind="ExternalInput")
    ad_d = nc.dram_tensor("adiag", [P, M], bf16, kind="ExternalInput")
    out_d = nc.dram_tensor("outr", [P, FREE], bf16, kind="ExternalOutput")

    ADD = mybir.AluOpType.add

    with tile.TileContext(nc) as tc:
        with (
            tc.tile_pool(name="big", bufs=2) as big,
            tc.tile_pool(name="cst", bufs=2) as cst,
            tc.tile_pool(name="scr", bufs=1) as scr,
            tc.tile_pool(name="aux", bufs=2) as aux,
            tc.tile_pool(name="ot", bufs=3) as otp,
            tc.tile_pool(name="pm", bufs=3, space="PSUM") as pmp,
            tc.tile_pool(name="pa", bufs=1, space="PSUM") as pap,
        ):
          for _rep in range(repeat):
            mm = nc.tensor.matmul

            # ---- x loads first (critical path); stats overlap per slice ----
            xa = big.tile([P, FREE], bf16)
            xa_ap = xa[:]

            def xap(offset, dims):
                return bass.AP(
                    tensor=xa_ap.tensor,
                    offset=xa_ap.offset + offset,
                    ap=[list(xa_ap.ap[0])] + dims,
                )

            def vap(t, offset, dims):
                a = t[:]
                return bass.AP(tensor=a.tensor, offset=a.offset + offset,
                               ap=[list(a.ap[0])] + dims)

            def stt_add(out, in0, in1):
                nc.vector.scalar_tensor_tensor(out=out, in0=in0, scalar=0.0,
                                               in1=in1, op0=ADD, op1=ADD)

            # stats scratch (bufs=1; consumed within the rep's stats phase)
            rs_a = scr.tile([P, 2048], bf16)
            rs_b = scr.tile([P, 1024], bf16)
            rs_c = scr.tile([P, 512], bf16)
            rows8 = scr.tile([P, 1024], bf16)   # [i(128), 8] row partials
            rows4 = scr.tile([P, 512], bf16)
            rows2 = scr.tile([P, 256], bf16)
            cs_a = scr.tile([P, 2048], bf16)
            cs_b = scr.tile([P, 1024], bf16)
            cs_c = scr.tile([P, 512], bf16)
            cs_d = scr.tile([P, 256], bf16)
            cs_t = scr.tile([P, 128], bf16)
            sd_f = scr.tile([P, 1], f32)
            tot_f = scr.tile([P, 1], f32)

            # stats kept across the rep (read by aux matmuls)
            rowsum = aux.tile([P, M], bf16)
            colsum = aux.tile([P, M], bf16)
            diagx = aux.tile([P, M], bf16)
            sd = aux.tile([P, 1], bf16)
            tot = aux.tile([P, 1], bf16)
            rowft = aux.tile([P, M], bf16)   # RowF^T: [i, q]
            colf = aux.tile([P, M], bf16)    # ColF + Const + bias: [q, j]
            diaf = aux.tile([P, M], bf16)    # DiagF + diag_bias: [q, i]

            for t in range(NLOAD):
                sl = slice(t * SL, (t + 1) * SL)
                nc.sync.dma_start(out=xa[:, sl], in_=xr_d[:, sl])
                off = t * SL
                # rowsum partials: fold j 128 -> 8 (4 stt levels, 4x mode)
                stt_add(vap(rs_a, 0, [[64, IPS], [1, 64]]),
                        xap(off, [[M, IPS], [1, 64]]),
                        xap(off + 64, [[M, IPS], [1, 64]]))
                stt_add(vap(rs_b, 0, [[32, IPS], [1, 32]]),
                        vap(rs_a, 0, [[64, IPS], [1, 32]]),
                        vap(rs_a, 32, [[64, IPS], [1, 32]]))
                stt_add(vap(rs_c, 0, [[16, IPS], [1, 16]]),
                        vap(rs_b, 0, [[32, IPS], [1, 16]]),
                        vap(rs_b, 16, [[32, IPS], [1, 16]]))
                stt_add(vap(rows8, t * IPS * 8, [[8, IPS], [1, 8]]),
                        vap(rs_c, 0, [[16, IPS], [1, 8]]),
                        vap(rs_c, 8, [[16, IPS], [1, 8]]))
                # colsum partials: fold i 32 -> 1 (5 levels), accumulate
                stt_add(cs_a[:], xap(off, [[1, 16 * M]]),
                        xap(off + 16 * M, [[1, 16 * M]]))
                stt_add(cs_b[:], vap(cs_a, 0, [[1, 8 * M]]),
                        vap(cs_a, 8 * M, [[1, 8 * M]]))
                stt_add(cs_c[:], vap(cs_b, 0, [[1, 4 * M]]),
                        vap(cs_b, 4 * M, [[1, 4 * M]]))
                stt_add(cs_d[:], vap(cs_c, 0, [[1, 2 * M]]),
                        vap(cs_c, 2 * M, [[1, 2 * M]]))
                if t == 0:
                    stt_add(colsum[:], vap(cs_d, 0, [[1, M]]),
                            vap(cs_d, M, [[1, M]]))
                else:
                    stt_add(cs_t[:], vap(cs_d, 0, [[1, M]]),
                            vap(cs_d, M, [[1, M]]))
                    stt_add(colsum[:], colsum[:], cs_t[:])

            # rowsum tail: fold 8 -> 1 over [i, 8] partials
            stt_add(vap(rows4, 0, [[4, M], [1, 4]]),
                    vap(rows8, 0, [[8, M], [1, 4]]),
                    vap(rows8, 4, [[8, M], [1, 4]]))
            stt_add(vap(rows2, 0, [[2, M], [1, 2]]),
                    vap(rows4, 0, [[4, M], [1, 2]]),
                    vap(rows4, 2, [[4, M], [1, 2]]))
            stt_add(rowsum[:], vap(rows2, 0, [[2, M]]),
                    vap(rows2, 1, [[2, M]]))
            # diag (strided copy), sd, tot
            nc.vector.tensor_copy(out=diagx[:], in_=xap(0, [[M + 1, M]]))
            nc.vector.reduce_sum(out=sd_f[:], in_=diagx[:],
                                 axis=mybir.AxisListType.X)
            nc.vector.reduce_sum(out=tot_f[:], in_=rowsum[:],
                                 axis=mybir.AxisListType.X)
            nc.vector.tensor_scalar(out=sd[:], in0=sd_f[:], scalar1=0.0,
                                    scalar2=None, op0=ADD)
            nc.vector.tensor_scalar(out=tot[:], in0=tot_f[:], scalar1=0.0,
                                    scalar2=None, op0=ADD)

            # ---- constants (DMAs queue behind x on SP; needed only at aux) --
            wm = cst.tile([P, 15, P], bf16)
            nc.gpsimd.memset(wm[:], 0.0)
            for nq in range(NPC):
                nc.sync.dma_start(
                    out=wm[nq * D:(nq + 1) * D, :, nq * S:(nq + 1) * S],
                    in_=wm_d[:],
                )
            bc = cst.tile([P, 2], f32)
            nc.sync.dma_start(out=bc[:], in_=bc_d[:])
            adg = cst.tile([P, M], bf16)
            nc.sync.dma_start(out=adg[:], in_=ad_d[:])
            adg_ap = adg[:]

            W = lambda idx: wm[:, idx, :]
            (W_X, W_XT, W_ROW_CS, W_ROW_RS, W_ROW_DG, W_COL_CS, W_COL_RS,
             W_COL_DG, W_DIA_DG, W_DIA_RS, W_DIA_CS, W_SD_SD, W_SD_TOT,
             W_SC_SD, W_SC_TOT) = range(15)

            # ---- aux contractions over d (partition dim) on the PE ----
            pa = pap.tile([P, CHUNK], f32)
            # ColF raw [q, j] in pa[0:M]
            mm(pa[:, 0:M], W(W_COL_CS), colsum[:], start=True, stop=False)
            mm(pa[:, 0:M], W(W_COL_RS), rowsum[:], start=False, stop=False)
            mm(pa[:, 0:M], W(W_COL_DG), diagx[:], start=False, stop=True)
            # DiagT raw [q, i] in pa[M:2M]
            mm(pa[:, M:2 * M], W(W_DIA_DG), diagx[:], start=True, stop=False)
            mm(pa[:, M:2 * M], W(W_DIA_RS), rowsum[:], start=False, stop=False)
            mm(pa[:, M:2 * M], W(W_DIA_CS), colsum[:], start=False, stop=True)
            # diag-const [q,1] in pa[2M:2M+1]; grid-const [q,1] in pa[2M+1:2M+2]
            mm(pa[:, 2 * M:2 * M + 1], W(W_SD_SD), sd[:], start=True, stop=False)
            mm(pa[:, 2 * M:2 * M + 1], W(W_SD_TOT), tot[:], start=False, stop=True)
            mm(pa[:, 2 * M + 1:2 * M + 2], W(W_SC_SD), sd[:], start=True, stop=False)
            mm(pa[:, 2 * M + 1:2 * M + 2], W(W_SC_TOT), tot[:], start=False, stop=True)
            # RowF^T [i, q]: swapped operands (stats stationary, W moving)
            pr = pap.tile([P, CHUNK], f32)
            mm(pr[:, 0:M], colsum[:], W(W_ROW_CS), start=True, stop=False)
            mm(pr[:, 0:M], rowsum[:], W(W_ROW_RS), start=False, stop=False)
            mm(pr[:, 0:M], diagx[:], W(W_ROW_DG), start=False, stop=True)

            # folds: colf = ColF + Const + bias; diaf = DiagT + DiagConst + dbias
            nc.vector.tensor_scalar(out=colf[:], in0=pa[:, 0:M],
                                    scalar1=pa[:, 2 * M + 1:2 * M + 2],
                                    scalar2=bc[:, 0:1], op0=ADD, op1=ADD)
            nc.vector.tensor_scalar(out=diaf[:], in0=pa[:, M:2 * M],
                                    scalar1=pa[:, 2 * M:2 * M + 1],
                                    scalar2=bc[:, 1:2], op0=ADD, op1=ADD)
            nc.vector.tensor_copy(out=rowft[:], in_=pr[:, 0:M])

            # ---- main einsum + assembly, 8-chunk groups, bf16 out ----
            for g in range(NGROUP):
                ot = otp.tile([P, GW], bf16)
                for u in range(4):  # pairs of chunks -> one [P,1024] psum tile
                    pm2 = pmp.tile([P, 2 * CHUNK], f32, tag="pm")
                    for h in range(2):
                        c = g * 8 + u * 2 + h
                        ps = pm2[:, h * CHUNK:(h + 1) * CHUNK]
                        mm(ps, W(W_X), xa[:, c * CHUNK:(c + 1) * CHUNK],
                           start=True, stop=False)
                        mm(ps, W(W_XT), xap(4 * c, [[1, 4], [M, M]]),
                           start=False, stop=False)
                        mm(ps, rowft[:],
                           bass.AP(tensor=adg_ap.tensor,
                                   offset=adg_ap.offset + (M - 1) - 4 * c,
                                   ap=[list(adg_ap.ap[0]), [-1, 4], [0, M]]),
                           start=False, stop=True)
                    nc.scalar.copy(out=ot[:, u * 1024:(u + 1) * 1024],
                                   in_=pm2[:])
                # Col+Const+bias: one broadcast-AP add over the whole group
                otv = vap(ot, 0, [[M, GW // M], [1, M]])
                cfb = vap(colf, 0, [[0, GW // M], [1, M]])
                nc.vector.scalar_tensor_tensor(out=otv, in0=otv, scalar=0.0,
                                               in1=cfb, op0=ADD, op1=ADD)
                # diagonal: 32 sparse adds in one strided op
                dview = vap(ot, 32 * g, [[516, 8], [129, 4]])
                dsrc = vap(diaf, 32 * g, [[4, 8], [1, 4]])
                nc.vector.tensor_tensor(out=dview, in0=dview, in1=dsrc, op=ADD)
                nc.scalar.dma_start(out=out_d[:, g * GW:(g + 1) * GW], in_=ot[:])

    nc.compile()
    return nc


def _get_nc():
    if "nc" not in _cache:
        _cache["nc"] = _build_program()
    return _cache["nc"]


def _host_prep(coefs, bias, diag_bias):
    import ml_dtypes

    m = float(M)
    C = np.asarray(coefs, dtype=np.float32)

    def bd(b, scale=1.0):
        return C[:, :, b] * np.float32(scale)

    # [15, D, S] pre-scaled coef blocks -> transpose to compact [D, 15, S]
    wmats = np.stack([
        bd(9),              # W_X
        bd(10),             # W_XT
        bd(5, 1 / m),       # W_ROW_CS
        bd(6, 1 / m),       # W_ROW_RS
        bd(11),             # W_ROW_DG
        bd(7, 1 / m),       # W_COL_CS
        bd(8, 1 / m),       # W_COL_RS
        bd(12),             # W_COL_DG
        bd(0),              # W_DIA_DG
        bd(2, 1 / m),       # W_DIA_RS
        bd(3, 1 / m),       # W_DIA_CS
        bd(1, 1 / m),       # W_SD_SD
        bd(4, 1 / (m * m)),  # W_SD_TOT
        bd(13, 1 / m),      # W_SC_SD
        bd(14, 1 / (m * m)),  # W_SC_TOT
    ]).astype(np.float32)
    wmats = np.ascontiguousarray(
        wmats.transpose(1, 0, 2).astype(ml_dtypes.bfloat16))
    bcols = np.stack([
        np.tile(np.asarray(bias, np.float32).reshape(S), NPC),
        np.tile(np.asarray(diag_bias, np.float32).reshape(S), NPC),
    ], axis=1).astype(np.float32)
    return wmats, np.ascontiguousarray(bcols)


def _in_maps(inputs, coefs, bias, diag_bias):
    import ml_dtypes

    x = np.asarray(inputs, np.float32).astype(ml_dtypes.bfloat16)
    wmats, bcols = _host_prep(coefs, bias, diag_bias)
    adiag = np.zeros((P, M), dtype=ml_dtypes.bfloat16)
    for k in range(M):
        adiag[k, (M - 1) - k] = 1.0
    maps = []
    for i in range(NCORES):
        xr = x[i * NPC:(i + 1) * NPC].reshape(P, FREE)
        maps.append({"xr": np.ascontiguousarray(xr), "wmats": wmats,
                     "bcols": bcols, "adiag": adiag})
    return maps


def run(inputs, coefs, bias, diag_bias, **spmd_kwargs):
    """Run on the 8 NeuronCores; returns (output, BassKernelResults)."""
    from concourse.bass_utils import run_bass_kernel_spmd

    nc = _get_nc()
    maps = _in_maps(inputs, coefs, bias, diag_bias)
    res = run_bass_kernel_spmd(nc, maps, list(range(NCORES)), **spmd_kwargs)
    out = np.concatenate(
        [np.asarray(r["outr"]).astype(np.float32).reshape(NPC, S, M, M)
         for r in res.results], axis=0
    )
    return np.ascontiguousarray(out), res


def kernel(inputs, coefs, bias, diag_bias):
    out, _ = run(inputs, coefs, bias, diag_bias)
    return out


# revision 23
# speedup vs baseline: 87.9496x; 1.4040x over previous
"""Eq2to2 equivariant layer (Maron et al. 2-to-2 basis, 15 ops) as a Trainium2
Bass/Tile kernel, data-parallel over the batch axis N across 8 NeuronCores.

Math: the 15-basis contraction collapses to
  out[n,s] = sum_d C9[d,s]*x[n,d] + sum_d C10[d,s]*x[n,d]^T
           + Row[n,s,i] (bcast over j) + Col[n,s,j] (bcast over i)
           + delta_ij * DiagT[n,s,i] + Const[n,s] + bias[s] + delta_ij*diag_bias[s]
where Row/Col/DiagT/Const are small contractions of rowsum/colsum/diag/tot stats.

This version is memory-roofline oriented: all HBM traffic is bf16 (host casts
both ways; tolerance 2e-2 >> bf16 rounding), halving DMA bytes vs f32.
Per 512-wide output chunk the PE runs THREE accumulating matmuls:
  psum  = W9^T  @ x_chunk                  (contiguous read)
  psum += W10^T @ x^T-chunk                (strided AP read, no data movement)
  psum += rowft[4c:4c+4]^T @ OnesBlk       (row-broadcast term, contract dim 4)
ACT drains psum->SBUF (1024-wide), DVE adds Col+Const+bias with one
broadcast-AP op per 4096-wide group and the sparse diagonal term, then the
group DMAs out. Row/col stats are computed with DVE pair-fold trees
(scalar_tensor_tensor runs in 4x DVE mode on packed bf16; TensorReduce has no
fast mode). Weights load compact [D,15,S] and scatter into a block-diagonal
[128,15,128] on device.

Layout: each core takes 4 n's -> 128 SBUF partitions = (nq, d). Grids are flat
in the free dim (16384 bf16 per partition).
"""

import sys

import numpy as np

if "/opt/trn_rl_repo" not in sys.path:
    sys.path.insert(0, "/opt/trn_rl_repo")

N, D, S, B, M = 32, 32, 32, 15, 128
NCORES = 8
NPC = N // NCORES          # n's per core = 4
P = 128                    # partitions
FREE = M * M               # 16384
CHUNK = 512                # psum half-bank pair (f32)
NCHUNK = FREE // CHUNK     # 32
NLOAD = 4                  # xa load slices (1 MiB bf16 each)
SL = FREE // NLOAD         # 4096 elements (32 i-rows) per load slice
IPS = SL // M              # i-rows per slice = 32
NGROUP = 4                 # out staging groups
GW = FREE // NGROUP        # 4096 elements per group (8 chunks)

_cache: dict = {}

ADD = None  # set in _build_program


def _build_program(repeat=1):
    import concourse.bass as bass
    import concourse.tile as tile
    from concourse import bacc, mybir

    f32 = mybir.dt.float32
    bf16 = mybir.dt.bfloat16
    nc = bacc.Bacc("TRN2", target_bir_lowering=False, debug=False)

    xr_d = nc.dram_tensor("xr", [P, FREE], bf16, kind="ExternalInput")
    # compact pre-scaled coefs [D, 15, S]; blockdiag scatter happens on-device
    wm_d = nc.dram_tensor("wmats", [D, 15, S], bf16, kind="ExternalInput")
    bc_d = nc.dram_tensor("bcols", [P, 2], f32, kind="ExternalInput")
    ad_d = nc.dram_tensor("adiag", [P, M], bf16, kind="ExternalInput")
    out_d = nc.dram_tensor("outr", [P, FREE], bf16, kind="ExternalOutput")

    ADD = mybir.AluOpType.add

    with tile.TileContext(nc) as tc:
        with (
            tc.tile_pool(name="big", bufs=2) as big,
            tc.tile_pool(name="cst", bufs=2) as cst,
            tc.tile_pool(name="scr", bufs=1) as scr,
            tc.tile_pool(name="aux", bufs=2) as aux,
            tc.tile_pool(name="ot", bufs=3) as otp,
            tc.tile_pool(name="pm", bufs=3, space="PSUM") as pmp,
            tc.tile_pool(name="pa", bufs=1, space="PSUM") as pap,
        ):
          for _rep in range(repeat):
            mm = nc.tensor.matmul

            # ---- x loads first (critical path); stats overlap per slice ----
            xa = big.tile([P, FREE], bf16)
            xa_ap = xa[:]

            def xap(offset, dims):
                return bass.AP(
                    tensor=xa_ap.tensor,
                    offset=xa_ap.offset + offset,
                    ap=[list(xa_ap.ap[0])] + dims,
                )

            def vap(t, offset, dims):
                a = t[:]
                return bass.AP(tensor=a.tensor, offset=a.offset + offset,
                               ap=[list(a.ap[0])] + dims)

            def stt_add(out, in0, in1, eng=None):
                (eng or nc.vector).tensor_tensor(out=out, in0=in0, in1=in1,
                                                 op=ADD)

            # stats scratch (bufs=1; consumed within the rep's stats phase)
            rs_a = scr.tile([P, 2048], bf16)
            rs_b = scr.tile([P, 1024], bf16)
            rs_c = scr.tile([P, 512], bf16)
            rows8 = scr.tile([P, 1024], bf16)   # [i(128), 8] row partials
            rows4 = scr.tile([P, 512], bf16)
            rows2 = scr.tile([P, 256], bf16)
            cs_a = scr.tile([P, 2048], bf16)    # Pool slices
            cs_a2 = scr.tile([P, 2048], bf16)   # DVE slices
            cs_b = scr.tile([P, 1024], bf16)
            cs_c = scr.tile([P, 512], bf16)
            cs_d = scr.tile([P, 256], bf16)
            cs_t = scr.tile([P, 128], bf16)
            sd_f = scr.tile([P, 1], f32)
            tot_f = scr.tile([P, 1], f32)

            # stats kept across the rep (read by aux matmuls)
            rowsum = aux.tile([P, M], bf16)
            colsum = aux.tile([P, M], bf16)
            diagx = aux.tile([P, M], bf16)
            sd = aux.tile([P, 1], bf16)
            tot = aux.tile([P, 1], bf16)
            rowft = aux.tile([P, M], bf16)   # RowF^T: [i, q]
            colf = aux.tile([P, M], bf16)    # ColF + Const + bias: [q, j]
            diaf = aux.tile([P, M], bf16)    # DiagF + diag_bias: [q, i]

            for t in range(NLOAD):
                sl = slice(t * SL, (t + 1) * SL)
                nc.sync.dma_start(out=xa[:, sl], in_=xr_d[:, sl])
                off = t * SL
                # rowsum partials: fold j 128 -> 8 (4 stt levels, 4x mode)
                stt_add(vap(rs_a, 0, [[64, IPS], [1, 64]]),
                        xap(off, [[M, IPS], [1, 64]]),
                        xap(off + 64, [[M, IPS], [1, 64]]))
                stt_add(vap(rs_b, 0, [[32, IPS], [1, 32]]),
                        vap(rs_a, 0, [[64, IPS], [1, 32]]),
                        vap(rs_a, 32, [[64, IPS], [1, 32]]))
                stt_add(vap(rs_c, 0, [[16, IPS], [1, 16]]),
                        vap(rs_b, 0, [[32, IPS], [1, 16]]),
                        vap(rs_b, 16, [[32, IPS], [1, 16]]))
                stt_add(vap(rows8, t * IPS * 8, [[8, IPS], [1, 8]]),
                        vap(rs_c, 0, [[16, IPS], [1, 8]]),
                        vap(rs_c, 8, [[16, IPS], [1, 8]]))
                # colsum partials: fold i 32 -> 1 (5 levels), accumulate.
                # First (biggest) level of the early slices runs on Pool,
                # which is otherwise idle during the load phase.
                csl1 = cs_a if t < 2 else cs_a2
                stt_add(csl1[:], xap(off, [[1, 16 * M]]),
                        xap(off + 16 * M, [[1, 16 * M]]),
                        eng=nc.gpsimd if t < 2 else None)
                stt_add(cs_b[:], vap(csl1, 0, [[1, 8 * M]]),
                        vap(csl1, 8 * M, [[1, 8 * M]]))
                stt_add(cs_c[:], vap(cs_b, 0, [[1, 4 * M]]),
                        vap(cs_b, 4 * M, [[1, 4 * M]]))
                stt_add(cs_d[:], vap(cs_c, 0, [[1, 2 * M]]),
                        vap(cs_c, 2 * M, [[1, 2 * M]]))
                if t == 0:
                    stt_add(colsum[:], vap(cs_d, 0, [[1, M]]),
                            vap(cs_d, M, [[1, M]]))
                else:
                    stt_add(cs_t[:], vap(cs_d, 0, [[1, M]]),
                            vap(cs_d, M, [[1, M]]))
                    stt_add(colsum[:], colsum[:], cs_t[:])

            # rowsum tail: fold 8 -> 1 over [i, 8] partials
            stt_add(vap(rows4, 0, [[4, M], [1, 4]]),
                    vap(rows8, 0, [[8, M], [1, 4]]),
                    vap(rows8, 4, [[8, M], [1, 4]]))
            stt_add(vap(rows2, 0, [[2, M], [1, 2]]),
                    vap(rows4, 0, [[4, M], [1, 2]]),
                    vap(rows4, 2, [[4, M], [1, 2]]))
            stt_add(rowsum[:], vap(rows2, 0, [[2, M]]),
                    vap(rows2, 1, [[2, M]]))
            # diag (strided copy), sd, tot
            nc.vector.tensor_copy(out=diagx[:], in_=xap(0, [[M + 1, M]]))
            nc.vector.reduce_sum(out=sd_f[:], in_=diagx[:],
                                 axis=mybir.AxisListType.X)
            nc.vector.reduce_sum(out=tot_f[:], in_=rowsum[:],
                                 axis=mybir.AxisListType.X)
            nc.vector.tensor_scalar(out=sd[:], in0=sd_f[:], scalar1=0.0,
                                    scalar2=None, op0=ADD)
            nc.vector.tensor_scalar(out=tot[:], in0=tot_f[:], scalar1=0.0,
                                    scalar2=None, op0=ADD)

            # ---- constants (DMAs queue behind x on SP; needed only at aux) --
            wm = cst.tile([P, 15, P], bf16)
            nc.gpsimd.memset(wm[:], 0.0)
            for nq in range(NPC):
                nc.sync.dma_start(
                    out=wm[nq * D:(nq + 1) * D, :, nq * S:(nq + 1) * S],
                    in_=wm_d[:],
                )
            bc = cst.tile([P, 2], f32)
            nc.sync.dma_start(out=bc[:], in_=bc_d[:])
            adg = cst.tile([P, M], bf16)
            nc.sync.dma_start(out=adg[:], in_=ad_d[:])
            adg_ap = adg[:]

            W = lambda idx: wm[:, idx, :]
            (W_X, W_XT, W_ROW_CS, W_ROW_RS, W_ROW_DG, W_COL_CS, W_COL_RS,
             W_COL_DG, W_DIA_DG, W_DIA_RS, W_DIA_CS, W_SD_SD, W_SD_TOT,
             W_SC_SD, W_SC_TOT) = range(15)

            # ---- aux contractions over d (partition dim) on the PE ----
            pa = pap.tile([P, CHUNK], f32)
            # ColF raw [q, j] in pa[0:M]
            mm(pa[:, 0:M], W(W_COL_CS), colsum[:], start=True, stop=False)
            mm(pa[:, 0:M], W(W_COL_RS), rowsum[:], start=False, stop=False)
            mm(pa[:, 0:M], W(W_COL_DG), diagx[:], start=False, stop=True)
            # DiagT raw [q, i] in pa[M:2M]
            mm(pa[:, M:2 * M], W(W_DIA_DG), diagx[:], start=True, stop=False)
            mm(pa[:, M:2 * M], W(W_DIA_RS), rowsum[:], start=False, stop=False)
            mm(pa[:, M:2 * M], W(W_DIA_CS), colsum[:], start=False, stop=True)
            # diag-const [q,1] in pa[2M:2M+1]; grid-const [q,1] in pa[2M+1:2M+2]
            mm(pa[:, 2 * M:2 * M + 1], W(W_SD_SD), sd[:], start=True, stop=False)
            mm(pa[:, 2 * M:2 * M + 1], W(W_SD_TOT), tot[:], start=False, stop=True)
            mm(pa[:, 2 * M + 1:2 * M + 2], W(W_SC_SD), sd[:], start=True, stop=False)
            mm(pa[:, 2 * M + 1:2 * M + 2], W(W_SC_TOT), tot[:], start=False, stop=True)
            # RowF^T [i, q]: swapped operands (stats stationary, W moving)
            pr = pap.tile([P, CHUNK], f32)
            mm(pr[:, 0:M], colsum[:], W(W_ROW_CS), start=True, stop=False)
            mm(pr[:, 0:M], rowsum[:], W(W_ROW_RS), start=False, stop=False)
            mm(pr[:, 0:M], diagx[:], W(W_ROW_DG), start=False, stop=True)

            # folds: colf = ColF + Const + bias; diaf = DiagT + DiagConst + dbias
            nc.vector.tensor_scalar(out=colf[:], in0=pa[:, 0:M],
                                    scalar1=pa[:, 2 * M + 1:2 * M + 2],
                                    scalar2=bc[:, 0:1], op0=ADD, op1=ADD)
            nc.vector.tensor_scalar(out=diaf[:], in0=pa[:, M:2 * M],
                                    scalar1=pa[:, 2 * M:2 * M + 1],
                                    scalar2=bc[:, 1:2], op0=ADD, op1=ADD)
            nc.scalar.copy(out=rowft[:], in_=pr[:, 0:M])

            # ---- main einsum + assembly, 8-chunk groups, bf16 out ----
            for g in range(NGROUP):
                ot = otp.tile([P, GW], bf16)
                for u in range(4):  # pairs of chunks -> one [P,1024] psum tile
                    pm2 = pmp.tile([P, 2 * CHUNK], f32, tag="pm")
                    for h in range(2):
                        c = g * 8 + u * 2 + h
                        ps = pm2[:, h * CHUNK:(h + 1) * CHUNK]
                        mm(ps, W(W_X), xa[:, c * CHUNK:(c + 1) * CHUNK],
                           start=True, stop=False)
                        mm(ps, W(W_XT), xap(4 * c, [[1, 4], [M, M]]),
                           start=False, stop=False)
                        mm(ps, rowft[:],
                           bass.AP(tensor=adg_ap.tensor,
                                   offset=adg_ap.offset + (M - 1) - 4 * c,
                                   ap=[list(adg_ap.ap[0]), [-1, 4], [0, M]]),
                           start=False, stop=True)
                    nc.scalar.copy(out=ot[:, u * 1024:(u + 1) * 1024],
                                   in_=pm2[:])
                # Col+Const+bias via broadcast-AP adds: Pool takes the first
                # pair (it is idle in the store phase), DVE the other three
                otv0 = vap(ot, 0, [[M, 8], [1, M]])
                cfb0 = vap(colf, 0, [[0, 8], [1, M]])
                nc.gpsimd.tensor_tensor(out=otv0, in0=otv0, in1=cfb0, op=ADD)
                otv = vap(ot, 1024, [[M, 24], [1, M]])
                cfb = vap(colf, 0, [[0, 24], [1, M]])
                nc.vector.tensor_tensor(out=otv, in0=otv, in1=cfb, op=ADD)
                # diagonal: 32 sparse adds in one strided op
                dview = vap(ot, 32 * g, [[516, 8], [129, 4]])
                dsrc = vap(diaf, 32 * g, [[4, 8], [1, 4]])
                nc.vector.tensor_tensor(out=dview, in0=dview, in1=dsrc, op=ADD)
                nc.scalar.dma_start(out=out_d[:, g * GW:(g + 1) * GW], in_=ot[:])

    nc.compile()
    return nc


def _get_nc():
    if "nc" not in _cache:
        _cache["nc"] = _build_program()
    return _cache["nc"]


def _host_prep(coefs, bias, diag_bias):
    import ml_dtypes

    m = float(M)
    C = np.asarray(coefs, dtype=np.float32)

    def bd(b, scale=1.0):
        return C[:, :, b] * np.float32(scale)

    # [15, D, S] pre-scaled coef blocks -> transpose to compact [D, 15, S]
    wmats = np.stack([
        bd(9),              # W_X
        bd(10),             # W_XT
        bd(5, 1 / m),       # W_ROW_CS
        bd(6, 1 / m),       # W_ROW_RS
        bd(11),             # W_ROW_DG
        bd(7, 1 / m),       # W_COL_CS
        bd(8, 1 / m),       # W_COL_RS
        bd(12),             # W_COL_DG
        bd(0),              # W_DIA_DG
        bd(2, 1 / m),       # W_DIA_RS
        bd(3, 1 / m),       # W_DIA_CS
        bd(1, 1 / m),       # W_SD_SD
        bd(4, 1 / (m * m)),  # W_SD_TOT
        bd(13, 1 / m),      # W_SC_SD
        bd(14, 1 / (m * m)),  # W_SC_TOT
    ]).astype(np.float32)
    wmats = np.ascontiguousarray(
        wmats.transpose(1, 0, 2).astype(ml_dtypes.bfloat16))
    bcols = np.stack([
        np.tile(np.asarray(bias, np.float32).reshape(S), NPC),
        np.tile(np.asarray(diag_bias, np.float32).reshape(S), NPC),
    ], axis=1).astype(np.float32)
    return wmats, np.ascontiguousarray(bcols)


def _in_maps(inputs, coefs, bias, diag_bias):
    import ml_dtypes

    x = np.asarray(inputs, np.float32).astype(ml_dtypes.bfloat16)
    wmats, bcols = _host_prep(coefs, bias, diag_bias)
    adiag = np.zeros((P, M), dtype=ml_dtypes.bfloat16)
    for k in range(M):
        adiag[k, (M - 1) - k] = 1.0
    maps = []
    for i in range(NCORES):
        xr = x[i * NPC:(i + 1) * NPC].reshape(P, FREE)
        maps.append({"xr": np.ascontiguousarray(xr), "wmats": wmats,
                     "bcols": bcols, "adiag": adiag})
    return maps


def run(inputs, coefs, bias, diag_bias, **spmd_kwargs):
    """Run on the 8 NeuronCores; returns (output, BassKernelResults)."""
    from concourse.bass_utils import run_bass_kernel_spmd

    nc = _get_nc()
    maps = _in_maps(inputs, coefs, bias, diag_bias)
    res = run_bass_kernel_spmd(nc, maps, list(range(NCORES)), **spmd_kwargs)
    out = np.concatenate(
        [np.asarray(r["outr"]).astype(np.float32).reshape(NPC, S, M, M)
         for r in res.results], axis=0
    )
    return np.ascontiguousarray(out), res


def kernel(inputs, coefs, bias, diag_bias):
    out, _ = run(inputs, coefs, bias, diag_bias)
    return out


# revision 30
# speedup vs baseline: 102.4867x; 1.1653x over previous
"""Eq2to2 equivariant layer (Maron et al. 2-to-2 basis, 15 ops) as a Trainium2
Bass/Tile kernel, data-parallel over the batch axis N across 8 NeuronCores.

Math: the 15-basis contraction collapses to
  out[n,s] = sum_d C9[d,s]*x[n,d] + sum_d C10[d,s]*x[n,d]^T
           + Row[n,s,i] (bcast over j) + Col[n,s,j] (bcast over i)
           + delta_ij * DiagT[n,s,i] + Const[n,s] + bias[s] + delta_ij*diag_bias[s]
where Row/Col/DiagT/Const are small contractions of rowsum/colsum/diag/tot stats.

This version is memory-roofline oriented: all HBM traffic is bf16 (host casts
both ways; tolerance 2e-2 >> bf16 rounding), halving DMA bytes vs f32.
Per 512-wide output chunk the PE runs THREE accumulating matmuls:
  psum  = W9^T  @ x_chunk                  (contiguous read)
  psum += W10^T @ x^T-chunk                (strided AP read, no data movement)
  psum += rowft[4c:4c+4]^T @ OnesBlk       (row-broadcast term, contract dim 4)
ACT drains psum->SBUF (1024-wide), DVE adds Col+Const+bias with one
broadcast-AP op per 4096-wide group and the sparse diagonal term, then the
group DMAs out. Row/col stats are computed with DVE pair-fold trees
(scalar_tensor_tensor runs in 4x DVE mode on packed bf16; TensorReduce has no
fast mode). Weights load compact [D,15,S] and scatter into a block-diagonal
[128,15,128] on device.

Layout: each core takes 4 n's -> 128 SBUF partitions = (nq, d). Grids are flat
in the free dim (16384 bf16 per partition).
"""

import sys

import numpy as np

if "/opt/trn_rl_repo" not in sys.path:
    sys.path.insert(0, "/opt/trn_rl_repo")

N, D, S, B, M = 32, 32, 32, 15, 128
NCORES = 8
NPC = N // NCORES          # n's per core = 4
P = 128                    # partitions
FREE = M * M               # 16384
CHUNK = 512                # psum half-bank pair (f32)
NCHUNK = FREE // CHUNK     # 32
NLOAD = 4                  # xa load slices (1 MiB bf16 each)
SL = FREE // NLOAD         # 4096 elements (32 i-rows) per load slice
IPS = SL // M              # i-rows per slice = 32
NGROUP = 4                 # out staging groups
GW = FREE // NGROUP        # 4096 elements per group (8 chunks)

_cache: dict = {}

ADD = None  # set in _build_program


def _build_program(repeat=1):
    import concourse.bass as bass
    import concourse.tile as tile
    from concourse import bacc, mybir

    f32 = mybir.dt.float32
    bf16 = mybir.dt.bfloat16
    nc = bacc.Bacc("TRN2", target_bir_lowering=False, debug=False)

    xr_d = nc.dram_tensor("xr", [P, FREE], bf16, kind="ExternalInput")
    # compact pre-scaled coefs [D, 15, S]; blockdiag scatter happens on-device
    wm_d = nc.dram_tensor("wmats", [D, 15, S], bf16, kind="ExternalInput")
    bc_d = nc.dram_tensor("bcols", [P, 2], f32, kind="ExternalInput")
    ad_d = nc.dram_tensor("adiag", [P, M], bf16, kind="ExternalInput")
    out_d = nc.dram_tensor("outr", [P, FREE], bf16, kind="ExternalOutput")

    ADD = mybir.AluOpType.add

    with tile.TileContext(nc) as tc:
        with (
            tc.tile_pool(name="big", bufs=2) as big,
            tc.tile_pool(name="cst", bufs=2) as cst,
            tc.tile_pool(name="scr", bufs=1) as scr,
            tc.tile_pool(name="aux", bufs=2) as aux,
            tc.tile_pool(name="ot", bufs=3) as otp,
            tc.tile_pool(name="pm", bufs=3, space="PSUM") as pmp,
            tc.tile_pool(name="pa", bufs=1, space="PSUM") as pap,
        ):
          for _rep in range(repeat):
            mm = nc.tensor.matmul

            # ---- x loads first (critical path); stats overlap per slice ----
            xa = big.tile([P, FREE], bf16)
            xa_ap = xa[:]

            def xap(offset, dims):
                return bass.AP(
                    tensor=xa_ap.tensor,
                    offset=xa_ap.offset + offset,
                    ap=[list(xa_ap.ap[0])] + dims,
                )

            def vap(t, offset, dims):
                a = t[:]
                return bass.AP(tensor=a.tensor, offset=a.offset + offset,
                               ap=[list(a.ap[0])] + dims)

            def stt_add(out, in0, in1, eng=None):
                (eng or nc.vector).tensor_tensor(out=out, in0=in0, in1=in1,
                                                 op=ADD)

            # stats scratch (bufs=1; consumed within the rep's stats phase)
            rs_a = scr.tile([P, 2048], bf16)
            rs_b = scr.tile([P, 1024], bf16)
            rs_c = scr.tile([P, 512], bf16)
            rows8 = scr.tile([P, 1024], bf16)   # [i(128), 8] row partials
            rows4 = scr.tile([P, 512], bf16)
            rows2 = scr.tile([P, 256], bf16)
            cs_a = scr.tile([P, 2048], bf16)    # Pool slices
            cs_a2 = scr.tile([P, 2048], bf16)   # DVE slices
            cs_b = scr.tile([P, 1024], bf16)
            cs_b2 = scr.tile([P, 1024], bf16)
            cs_c = scr.tile([P, 512], bf16)
            cs_d = scr.tile([P, 256], bf16)
            cs_t = scr.tile([P, 128], bf16)
            sd_f = scr.tile([P, 1], f32)
            tot_f = scr.tile([P, 1], f32)

            # stats kept across the rep (read by aux matmuls)
            rowsum = aux.tile([P, M], bf16)
            colsum = aux.tile([P, M], bf16)
            diagx = aux.tile([P, M], bf16)
            sd = aux.tile([P, 1], bf16)
            tot = aux.tile([P, 1], bf16)
            rowft = aux.tile([P, M], bf16)   # RowF^T: [i, q]
            colf = aux.tile([P, M], bf16)    # ColF + Const + bias: [q, j]
            diaf = aux.tile([P, M], bf16)    # DiagF + diag_bias: [q, i]

            # constants go on the ACT queue so they overlap the x loads and
            # never delay the aux matmuls
            wm = cst.tile([P, 15, P], bf16)
            nc.gpsimd.memset(wm[:], 0.0)
            for nq in range(NPC):
                nc.scalar.dma_start(
                    out=wm[nq * D:(nq + 1) * D, :, nq * S:(nq + 1) * S],
                    in_=wm_d[:],
                )
            bc = cst.tile([P, 2], f32)
            nc.scalar.dma_start(out=bc[:], in_=bc_d[:])
            adg = cst.tile([P, M], bf16)
            nc.scalar.dma_start(out=adg[:], in_=ad_d[:])
            adg_ap = adg[:]

            for t in range(NLOAD):
                sl = slice(t * SL, (t + 1) * SL)
                nc.sync.dma_start(out=xa[:, sl], in_=xr_d[:, sl])
                off = t * SL
                # diag elements of this slice (i in [32t, 32t+32))
                nc.vector.tensor_copy(
                    out=diagx[:, t * IPS:(t + 1) * IPS],
                    in_=xap(4128 * t, [[M + 1, IPS]]))
                # rowsum partials: fold j 128 -> 8 (4 stt levels, 4x mode)
                stt_add(vap(rs_a, 0, [[64, IPS], [1, 64]]),
                        xap(off, [[M, IPS], [1, 64]]),
                        xap(off + 64, [[M, IPS], [1, 64]]))
                stt_add(vap(rs_b, 0, [[32, IPS], [1, 32]]),
                        vap(rs_a, 0, [[64, IPS], [1, 32]]),
                        vap(rs_a, 32, [[64, IPS], [1, 32]]))
                stt_add(vap(rs_c, 0, [[16, IPS], [1, 16]]),
                        vap(rs_b, 0, [[32, IPS], [1, 16]]),
                        vap(rs_b, 16, [[32, IPS], [1, 16]]))
                stt_add(vap(rows8, t * IPS * 8, [[8, IPS], [1, 8]]),
                        vap(rs_c, 0, [[16, IPS], [1, 8]]),
                        vap(rs_c, 8, [[16, IPS], [1, 8]]))
                # colsum partials: fold i 32 -> 1 (5 levels), accumulate.
                # Pool (otherwise idle during loads) handles slices 0-2
                # entirely; DVE takes the last slice so both finish ~together.
                eng = nc.gpsimd if t < 3 else nc.vector
                csl1 = cs_a if t < 3 else cs_a2
                csl2 = cs_b if t < 3 else cs_b2
                stt_add(csl1[:], xap(off, [[1, 16 * M]]),
                        xap(off + 16 * M, [[1, 16 * M]]), eng=eng)
                stt_add(csl2[:], vap(csl1, 0, [[1, 8 * M]]),
                        vap(csl1, 8 * M, [[1, 8 * M]]), eng=eng)
                stt_add(cs_c[:], vap(csl2, 0, [[1, 4 * M]]),
                        vap(csl2, 4 * M, [[1, 4 * M]]), eng=eng)
                stt_add(cs_d[:], vap(cs_c, 0, [[1, 2 * M]]),
                        vap(cs_c, 2 * M, [[1, 2 * M]]), eng=eng)
                if t == 0:
                    stt_add(colsum[:], vap(cs_d, 0, [[1, M]]),
                            vap(cs_d, M, [[1, M]]), eng=eng)
                else:
                    stt_add(cs_t[:], vap(cs_d, 0, [[1, M]]),
                            vap(cs_d, M, [[1, M]]), eng=eng)
                    stt_add(colsum[:], colsum[:], cs_t[:], eng=eng)

            # rowsum tail: fold 8 -> 1 over [i, 8] partials
            stt_add(vap(rows4, 0, [[4, M], [1, 4]]),
                    vap(rows8, 0, [[8, M], [1, 4]]),
                    vap(rows8, 4, [[8, M], [1, 4]]))
            stt_add(vap(rows2, 0, [[2, M], [1, 2]]),
                    vap(rows4, 0, [[4, M], [1, 2]]),
                    vap(rows4, 2, [[4, M], [1, 2]]))
            stt_add(rowsum[:], vap(rows2, 0, [[2, M]]),
                    vap(rows2, 1, [[2, M]]))
            # sd, tot
            nc.vector.reduce_sum(out=sd_f[:], in_=diagx[:],
                                 axis=mybir.AxisListType.X)
            nc.vector.reduce_sum(out=tot_f[:], in_=rowsum[:],
                                 axis=mybir.AxisListType.X)
            nc.vector.tensor_scalar(out=sd[:], in0=sd_f[:], scalar1=0.0,
                                    scalar2=None, op0=ADD)
            nc.vector.tensor_scalar(out=tot[:], in0=tot_f[:], scalar1=0.0,
                                    scalar2=None, op0=ADD)

            W = lambda idx: wm[:, idx, :]
            (W_X, W_XT, W_ROW_CS, W_ROW_RS, W_ROW_DG, W_COL_CS, W_COL_RS,
             W_COL_DG, W_DIA_DG, W_DIA_RS, W_DIA_CS, W_SD_SD, W_SD_TOT,
             W_SC_SD, W_SC_TOT) = range(15)

            # ---- aux contractions over d (partition dim) on the PE ----
            # RowF^T [i, q] first (it gates the main matmuls): swapped
            # operands (stats stationary, W moving); colsum arrives last so
            # its matmul closes each accumulation group.
            pr = pap.tile([P, CHUNK], f32)
            mm(pr[:, 0:M], rowsum[:], W(W_ROW_RS), start=True, stop=False)
            mm(pr[:, 0:M], diagx[:], W(W_ROW_DG), start=False, stop=False)
            mm(pr[:, 0:M], colsum[:], W(W_ROW_CS), start=False, stop=True)
            pa = pap.tile([P, CHUNK], f32)
            # ColF raw [q, j] in pa[0:M]
            mm(pa[:, 0:M], W(W_COL_RS), rowsum[:], start=True, stop=False)
            mm(pa[:, 0:M], W(W_COL_DG), diagx[:], start=False, stop=False)
            mm(pa[:, 0:M], W(W_COL_CS), colsum[:], start=False, stop=True)
            # DiagT raw [q, i] in pa[M:2M]
            mm(pa[:, M:2 * M], W(W_DIA_DG), diagx[:], start=True, stop=False)
            mm(pa[:, M:2 * M], W(W_DIA_RS), rowsum[:], start=False, stop=False)
            mm(pa[:, M:2 * M], W(W_DIA_CS), colsum[:], start=False, stop=True)
            # diag-const [q,1] in pa[2M:2M+1]; grid-const [q,1] in pa[2M+1:2M+2]
            mm(pa[:, 2 * M:2 * M + 1], W(W_SD_SD), sd[:], start=True, stop=False)
            mm(pa[:, 2 * M:2 * M + 1], W(W_SD_TOT), tot[:], start=False, stop=True)
            mm(pa[:, 2 * M + 1:2 * M + 2], W(W_SC_SD), sd[:], start=True, stop=False)
            mm(pa[:, 2 * M + 1:2 * M + 2], W(W_SC_TOT), tot[:], start=False, stop=True)

            # folds: colf = ColF + Const + bias; diaf = DiagT + DiagConst + dbias
            nc.vector.tensor_scalar(out=colf[:], in0=pa[:, 0:M],
                                    scalar1=pa[:, 2 * M + 1:2 * M + 2],
                                    scalar2=bc[:, 0:1], op0=ADD, op1=ADD)
            nc.vector.tensor_scalar(out=diaf[:], in0=pa[:, M:2 * M],
                                    scalar1=pa[:, 2 * M:2 * M + 1],
                                    scalar2=bc[:, 1:2], op0=ADD, op1=ADD)
            nc.scalar.copy(out=rowft[:], in_=pr[:, 0:M])

            # ---- main einsum + assembly, 8-chunk groups, bf16 out ----
            for g in range(NGROUP):
                ot = otp.tile([P, GW], bf16)
                for u in range(4):  # pairs of chunks -> one [P,1024] psum tile
                    pm2 = pmp.tile([P, 2 * CHUNK], f32, tag="pm")
                    for h in range(2):
                        c = g * 8 + u * 2 + h
                        ps = pm2[:, h * CHUNK:(h + 1) * CHUNK]
                        mm(ps, W(W_X), xa[:, c * CHUNK:(c + 1) * CHUNK],
                           start=True, stop=False)
                        mm(ps, W(W_XT), xap(4 * c, [[1, 4], [M, M]]),
                           start=False, stop=False)
                        mm(ps, rowft[:],
                           bass.AP(tensor=adg_ap.tensor,
                                   offset=adg_ap.offset + (M - 1) - 4 * c,
                                   ap=[list(adg_ap.ap[0]), [-1, 4], [0, M]]),
                           start=False, stop=True)
                    nc.scalar.copy(out=ot[:, u * 1024:(u + 1) * 1024],
                                   in_=pm2[:])
                # Col+Const+bias via broadcast-AP adds: Pool takes the first
                # pair (it is idle in the store phase), DVE the other three
                otv0 = vap(ot, 0, [[M, 8], [1, M]])
                cfb0 = vap(colf, 0, [[0, 8], [1, M]])
                nc.gpsimd.tensor_tensor(out=otv0, in0=otv0, in1=cfb0, op=ADD)
                otv = vap(ot, 1024, [[M, 24], [1, M]])
                cfb = vap(colf, 0, [[0, 24], [1, M]])
                nc.vector.tensor_tensor(out=otv, in0=otv, in1=cfb, op=ADD)
                # diagonal: 32 sparse adds in one strided op
                dview = vap(ot, 32 * g, [[516, 8], [129, 4]])
                dsrc = vap(diaf, 32 * g, [[4, 8], [1, 4]])
                nc.vector.tensor_tensor(out=dview, in0=dview, in1=dsrc, op=ADD)
                nc.gpsimd.dma_start(out=out_d[:, g * GW:(g + 1) * GW], in_=ot[:])

    nc.compile()
    return nc


def _get_nc():
    if "nc" not in _cache:
        _cache["nc"] = _build_program()
    return _cache["nc"]


def _host_prep(coefs, bias, diag_bias):
    import ml_dtypes

    m = float(M)
    C = np.asarray(coefs, dtype=np.float32)

    def bd(b, scale=1.0):
        return C[:, :, b] * np.float32(scale)

    # [15, D, S] pre-scaled coef blocks -> transpose to compact [D, 15, S]
    wmats = np.stack([
        bd(9),              # W_X
        bd(10),             # W_XT
        bd(5, 1 / m),       # W_ROW_CS
        bd(6, 1 / m),       # W_ROW_RS
        bd(11),             # W_ROW_DG
        bd(7, 1 / m),       # W_COL_CS
        bd(8, 1 / m),       # W_COL_RS
        bd(12),             # W_COL_DG
        bd(0),              # W_DIA_DG
        bd(2, 1 / m),       # W_DIA_RS
        bd(3, 1 / m),       # W_DIA_CS
        bd(1, 1 / m),       # W_SD_SD
        bd(4, 1 / (m * m)),  # W_SD_TOT
        bd(13, 1 / m),      # W_SC_SD
        bd(14, 1 / (m * m)),  # W_SC_TOT
    ]).astype(np.float32)
    wmats = np.ascontiguousarray(
        wmats.transpose(1, 0, 2).astype(ml_dtypes.bfloat16))
    bcols = np.stack([
        np.tile(np.asarray(bias, np.float32).reshape(S), NPC),
        np.tile(np.asarray(diag_bias, np.float32).reshape(S), NPC),
    ], axis=1).astype(np.float32)
    return wmats, np.ascontiguousarray(bcols)


def _in_maps(inputs, coefs, bias, diag_bias):
    import ml_dtypes

    x = np.asarray(inputs, np.float32).astype(ml_dtypes.bfloat16)
    wmats, bcols = _host_prep(coefs, bias, diag_bias)
    adiag = np.zeros((P, M), dtype=ml_dtypes.bfloat16)
    for k in range(M):
        adiag[k, (M - 1) - k] = 1.0
    maps = []
    for i in range(NCORES):
        xr = x[i * NPC:(i + 1) * NPC].reshape(P, FREE)
        maps.append({"xr": np.ascontiguousarray(xr), "wmats": wmats,
                     "bcols": bcols, "adiag": adiag})
    return maps


def run(inputs, coefs, bias, diag_bias, **spmd_kwargs):
    """Run on the 8 NeuronCores; returns (output, BassKernelResults)."""
    from concourse.bass_utils import run_bass_kernel_spmd

    nc = _get_nc()
    maps = _in_maps(inputs, coefs, bias, diag_bias)
    res = run_bass_kernel_spmd(nc, maps, list(range(NCORES)), **spmd_kwargs)
    out = np.concatenate(
        [np.asarray(r["outr"]).astype(np.float32).reshape(NPC, S, M, M)
         for r in res.results], axis=0
    )
    return np.ascontiguousarray(out), res


def kernel(inputs, coefs, bias, diag_bias):
    out, _ = run(inputs, coefs, bias, diag_bias)
    return out


# revision 40
# speedup vs baseline: 103.3450x; 1.0084x over previous
"""Eq2to2 equivariant layer (Maron et al. 2-to-2 basis, 15 ops) as a Trainium2
Bass/Tile kernel, data-parallel over the batch axis N across 8 NeuronCores.

Math: the 15-basis contraction collapses to
  out[n,s] = sum_d C9[d,s]*x[n,d] + sum_d C10[d,s]*x[n,d]^T
           + Row[n,s,i] (bcast over j) + Col[n,s,j] (bcast over i)
           + delta_ij * DiagT[n,s,i] + Const[n,s] + bias[s] + delta_ij*diag_bias[s]
where Row/Col/DiagT/Const are small contractions of rowsum/colsum/diag/tot stats.

This version is memory-roofline oriented: all HBM traffic is bf16 (host casts
both ways; tolerance 2e-2 >> bf16 rounding), halving DMA bytes vs f32.
Per 512-wide output chunk the PE runs THREE accumulating matmuls:
  psum  = W9^T  @ x_chunk                  (contiguous read)
  psum += W10^T @ x^T-chunk                (strided AP read, no data movement)
  psum += rowft[4c:4c+4]^T @ OnesBlk       (row-broadcast term, contract dim 4)
ACT drains psum->SBUF (1024-wide), DVE adds Col+Const+bias with one
broadcast-AP op per 4096-wide group and the sparse diagonal term, then the
group DMAs out. Row/col stats are computed with DVE pair-fold trees
(scalar_tensor_tensor runs in 4x DVE mode on packed bf16; TensorReduce has no
fast mode). Weights load compact [D,15,S] and scatter into a block-diagonal
[128,15,128] on device.

Layout: each core takes 4 n's -> 128 SBUF partitions = (nq, d). Grids are flat
in the free dim (16384 bf16 per partition).
"""

import sys

import numpy as np

if "/opt/trn_rl_repo" not in sys.path:
    sys.path.insert(0, "/opt/trn_rl_repo")

N, D, S, B, M = 32, 32, 32, 15, 128
NCORES = 8
NPC = N // NCORES          # n's per core = 4
P = 128                    # partitions
FREE = M * M               # 16384
CHUNK = 512                # psum half-bank pair (f32)
NCHUNK = FREE // CHUNK     # 32
NLOAD = 8                  # xa load slices (0.5 MiB bf16 each)
SL = FREE // NLOAD         # 2048 elements (16 i-rows) per load slice
IPS = SL // M              # i-rows per slice = 16
NGROUP = 4                 # out staging groups
GW = FREE // NGROUP        # 4096 elements per group (8 chunks)

_cache: dict = {}

ADD = None  # set in _build_program


def _build_program(repeat=1):
    import concourse.bass as bass
    import concourse.tile as tile
    from concourse import bacc, mybir

    f32 = mybir.dt.float32
    bf16 = mybir.dt.bfloat16
    nc = bacc.Bacc("TRN2", target_bir_lowering=False, debug=False)

    xr_d = nc.dram_tensor("xr", [P, FREE], bf16, kind="ExternalInput")
    # compact pre-scaled coefs [D, 15, S]; blockdiag scatter happens on-device
    wm_d = nc.dram_tensor("wmats", [D, 15, S], bf16, kind="ExternalInput")
    bc_d = nc.dram_tensor("bcols", [P, 2], f32, kind="ExternalInput")
    ad_d = nc.dram_tensor("adiag", [P, M], bf16, kind="ExternalInput")
    out_d = nc.dram_tensor("outr", [P, FREE], bf16, kind="ExternalOutput")

    ADD = mybir.AluOpType.add

    with tile.TileContext(nc) as tc:
        with (
            tc.tile_pool(name="big", bufs=2) as big,
            tc.tile_pool(name="cst", bufs=2) as cst,
            tc.tile_pool(name="scr", bufs=1) as scr,
            tc.tile_pool(name="aux", bufs=2) as aux,
            tc.tile_pool(name="ot", bufs=3) as otp,
            tc.tile_pool(name="pm", bufs=3, space="PSUM") as pmp,
            tc.tile_pool(name="pa", bufs=1, space="PSUM") as pap,
        ):
          for _rep in range(repeat):
            mm = nc.tensor.matmul

            # ---- x loads first (critical path); stats overlap per slice ----
            xa = big.tile([P, FREE], bf16)
            xa_ap = xa[:]

            def xap(offset, dims):
                return bass.AP(
                    tensor=xa_ap.tensor,
                    offset=xa_ap.offset + offset,
                    ap=[list(xa_ap.ap[0])] + dims,
                )

            def vap(t, offset, dims):
                a = t[:]
                return bass.AP(tensor=a.tensor, offset=a.offset + offset,
                               ap=[list(a.ap[0])] + dims)

            def stt_add(out, in0, in1, eng=None):
                (eng or nc.vector).tensor_tensor(out=out, in0=in0, in1=in1,
                                                 op=ADD)

            # stats scratch (bufs=1; consumed within the rep's stats phase)
            rs_a = scr.tile([P, SL // 2], bf16)
            rs_b = scr.tile([P, SL // 4], bf16)
            rs_c = scr.tile([P, SL // 8], bf16)
            rows8 = scr.tile([P, 1024], bf16)   # [i(128), 8] row partials
            rows4 = scr.tile([P, 512], bf16)
            rows2 = scr.tile([P, 256], bf16)
            # two full colsum scratch sets: one per engine (Pool / DVE)
            cs_a = scr.tile([P, SL // 2], bf16)
            cs_b = scr.tile([P, SL // 4], bf16)
            cs_c = scr.tile([P, SL // 8], bf16)
            cs_t = scr.tile([P, M], bf16)
            cs_a2 = scr.tile([P, SL // 2], bf16)
            cs_b2 = scr.tile([P, SL // 4], bf16)
            cs_c2 = scr.tile([P, SL // 8], bf16)
            cs_t2 = scr.tile([P, M], bf16)
            sd_f = scr.tile([P, 1], f32)
            tot_f = scr.tile([P, 1], f32)

            # stats kept across the rep (read by aux matmuls)
            rowsum = aux.tile([P, M], bf16)
            colsum = aux.tile([P, M], bf16)
            diagx = aux.tile([P, M], bf16)
            sd = aux.tile([P, 1], bf16)
            tot = aux.tile([P, 1], bf16)
            rowft = aux.tile([P, M], bf16)   # RowF^T: [i, q]
            rowfb = aux.tile([P, M], bf16)   # RowF: [q, i] (ACT-bias path)
            colf = aux.tile([P, M], bf16)    # ColF + Const + bias: [q, j]
            diaf = aux.tile([P, M], bf16)    # DiagF + diag_bias: [q, i]

            # constants go on the ACT queue so they overlap the x loads and
            # never delay the aux matmuls
            wm = cst.tile([P, 15, P], bf16)
            if _rep < 2:
                # pool rotates 2 wm tiles; later reps re-scatter the diagonal
                # blocks onto already-zeroed off-block space
                nc.gpsimd.memset(wm[:], 0.0)
            for nq in range(NPC):
                nc.scalar.dma_start(
                    out=wm[nq * D:(nq + 1) * D, :, nq * S:(nq + 1) * S],
                    in_=wm_d[:],
                )
            bc = cst.tile([P, 2], f32)
            nc.scalar.dma_start(out=bc[:], in_=bc_d[:])
            adg = cst.tile([P, M], bf16)
            nc.scalar.dma_start(out=adg[:], in_=ad_d[:])
            adg_ap = adg[:]

            for t in range(NLOAD):
                sl = slice(t * SL, (t + 1) * SL)
                nc.sync.dma_start(out=xa[:, sl], in_=xr_d[:, sl])
                off = t * SL
                # diag elements of this slice (i in [32t, 32t+32))
                nc.vector.tensor_copy(
                    out=diagx[:, t * IPS:(t + 1) * IPS],
                    in_=xap((M + 1) * IPS * t, [[M + 1, IPS]]))
                # rowsum partials: fold j 128 -> 8 (4 stt levels, 4x mode)
                stt_add(vap(rs_a, 0, [[64, IPS], [1, 64]]),
                        xap(off, [[M, IPS], [1, 64]]),
                        xap(off + 64, [[M, IPS], [1, 64]]))
                stt_add(vap(rs_b, 0, [[32, IPS], [1, 32]]),
                        vap(rs_a, 0, [[64, IPS], [1, 32]]),
                        vap(rs_a, 32, [[64, IPS], [1, 32]]))
                stt_add(vap(rs_c, 0, [[16, IPS], [1, 16]]),
                        vap(rs_b, 0, [[32, IPS], [1, 16]]),
                        vap(rs_b, 16, [[32, IPS], [1, 16]]))
                stt_add(vap(rows8, t * IPS * 8, [[8, IPS], [1, 8]]),
                        vap(rs_c, 0, [[16, IPS], [1, 8]]),
                        vap(rs_c, 8, [[16, IPS], [1, 8]]))
                # colsum partials: fold i 16 -> 1 (4 levels), accumulate.
                # Pool (otherwise idle during loads) takes the first 6
                # slices with its own scratch set; DVE takes the last two.
                pool_side = t < 6
                eng = nc.gpsimd if pool_side else nc.vector
                c1, c2, c3, ct = ((cs_a, cs_b, cs_c, cs_t) if pool_side
                                  else (cs_a2, cs_b2, cs_c2, cs_t2))
                stt_add(c1[:], xap(off, [[1, 8 * M]]),
                        xap(off + 8 * M, [[1, 8 * M]]), eng=eng)
                stt_add(c2[:], vap(c1, 0, [[1, 4 * M]]),
                        vap(c1, 4 * M, [[1, 4 * M]]), eng=eng)
                stt_add(c3[:], vap(c2, 0, [[1, 2 * M]]),
                        vap(c2, 2 * M, [[1, 2 * M]]), eng=eng)
                if t == 0:
                    stt_add(colsum[:], vap(c3, 0, [[1, M]]),
                            vap(c3, M, [[1, M]]), eng=eng)
                else:
                    stt_add(ct[:], vap(c3, 0, [[1, M]]),
                            vap(c3, M, [[1, M]]), eng=eng)
                    stt_add(colsum[:], colsum[:], ct[:], eng=eng)

            # rowsum tail: fold 8 -> 1 over [i, 8] partials
            stt_add(vap(rows4, 0, [[4, M], [1, 4]]),
                    vap(rows8, 0, [[8, M], [1, 4]]),
                    vap(rows8, 4, [[8, M], [1, 4]]))
            stt_add(vap(rows2, 0, [[2, M], [1, 2]]),
                    vap(rows4, 0, [[4, M], [1, 2]]),
                    vap(rows4, 2, [[4, M], [1, 2]]))
            stt_add(rowsum[:], vap(rows2, 0, [[2, M]]),
                    vap(rows2, 1, [[2, M]]))
            # sd, tot
            nc.vector.reduce_sum(out=sd_f[:], in_=diagx[:],
                                 axis=mybir.AxisListType.X)
            nc.vector.reduce_sum(out=tot_f[:], in_=rowsum[:],
                                 axis=mybir.AxisListType.X)
            nc.vector.tensor_scalar(out=sd[:], in0=sd_f[:], scalar1=0.0,
                                    scalar2=None, op0=ADD)
            nc.vector.tensor_scalar(out=tot[:], in0=tot_f[:], scalar1=0.0,
                                    scalar2=None, op0=ADD)

            W = lambda idx: wm[:, idx, :]
            (W_X, W_XT, W_ROW_CS, W_ROW_RS, W_ROW_DG, W_COL_CS, W_COL_RS,
             W_COL_DG, W_DIA_DG, W_DIA_RS, W_DIA_CS, W_SD_SD, W_SD_TOT,
             W_SC_SD, W_SC_TOT) = range(15)

            # ---- aux contractions over d (partition dim) on the PE ----
            # RowF^T [i, q] first (it gates the main matmuls): swapped
            # operands (stats stationary, W moving); colsum arrives last so
            # its matmul closes each accumulation group.
            pr = pap.tile([P, CHUNK], f32)
            mm(pr[:, 0:M], rowsum[:], W(W_ROW_RS), start=True, stop=False)
            mm(pr[:, 0:M], diagx[:], W(W_ROW_DG), start=False, stop=False)
            mm(pr[:, 0:M], colsum[:], W(W_ROW_CS), start=False, stop=True)
            # Row in [q, i] layout too (ACT-bias path for one pair per group)
            mm(pr[:, M:2 * M], W(W_ROW_RS), rowsum[:], start=True, stop=False)
            mm(pr[:, M:2 * M], W(W_ROW_DG), diagx[:], start=False, stop=False)
            mm(pr[:, M:2 * M], W(W_ROW_CS), colsum[:], start=False, stop=True)
            pa = pap.tile([P, CHUNK], f32)
            # ColF raw [q, j] in pa[0:M]
            mm(pa[:, 0:M], W(W_COL_RS), rowsum[:], start=True, stop=False)
            mm(pa[:, 0:M], W(W_COL_DG), diagx[:], start=False, stop=False)
            mm(pa[:, 0:M], W(W_COL_CS), colsum[:], start=False, stop=True)
            # DiagT raw [q, i] in pa[M:2M]
            mm(pa[:, M:2 * M], W(W_DIA_DG), diagx[:], start=True, stop=False)
            mm(pa[:, M:2 * M], W(W_DIA_RS), rowsum[:], start=False, stop=False)
            mm(pa[:, M:2 * M], W(W_DIA_CS), colsum[:], start=False, stop=True)
            # diag-const [q,1] in pa[2M:2M+1]; grid-const [q,1] in pa[2M+1:2M+2]
            mm(pa[:, 2 * M:2 * M + 1], W(W_SD_SD), sd[:], start=True, stop=False)
            mm(pa[:, 2 * M:2 * M + 1], W(W_SD_TOT), tot[:], start=False, stop=True)
            mm(pa[:, 2 * M + 1:2 * M + 2], W(W_SC_SD), sd[:], start=True, stop=False)
            mm(pa[:, 2 * M + 1:2 * M + 2], W(W_SC_TOT), tot[:], start=False, stop=True)

            # folds: colf = ColF + Const + bias; diaf = DiagT + DiagConst + dbias
            nc.vector.tensor_scalar(out=colf[:], in0=pa[:, 0:M],
                                    scalar1=pa[:, 2 * M + 1:2 * M + 2],
                                    scalar2=bc[:, 0:1], op0=ADD, op1=ADD)
            nc.vector.tensor_scalar(out=diaf[:], in0=pa[:, M:2 * M],
                                    scalar1=pa[:, 2 * M:2 * M + 1],
                                    scalar2=bc[:, 1:2], op0=ADD, op1=ADD)
            nc.scalar.copy(out=rowft[:], in_=pr[:, 0:M])
            nc.scalar.copy(out=rowfb[:], in_=pr[:, M:2 * M])

            # ---- main einsum + assembly, 8-chunk groups, bf16 out ----
            for g in range(NGROUP):
                ot = otp.tile([P, GW], bf16)
                for u in range(4):  # pairs of chunks -> one [P,1024] psum tile
                    pm2 = pmp.tile([P, 2 * CHUNK], f32, tag="pm")
                    act_row = u == 1  # row term via ACT bias for this pair
                    for h in range(2):
                        c = g * 8 + u * 2 + h
                        ps = pm2[:, h * CHUNK:(h + 1) * CHUNK]
                        mm(ps, W(W_X), xa[:, c * CHUNK:(c + 1) * CHUNK],
                           start=True, stop=False)
                        mm(ps, W(W_XT), xap(4 * c, [[1, 4], [M, M]]),
                           start=False, stop=act_row)
                        if not act_row:
                            mm(ps, rowft[:],
                               bass.AP(tensor=adg_ap.tensor,
                                       offset=adg_ap.offset + (M - 1) - 4 * c,
                                       ap=[list(adg_ap.ap[0]), [-1, 4], [0, M]]),
                               start=False, stop=True)
                    if act_row:
                        for h in range(2):
                            c = g * 8 + u * 2 + h
                            for qb in range(4):
                                o0 = u * 1024 + h * CHUNK + qb * M
                                nc.scalar.activation(
                                    out=ot[:, o0:o0 + M],
                                    in_=pm2[:, h * CHUNK + qb * M:
                                            h * CHUNK + (qb + 1) * M],
                                    func=mybir.ActivationFunctionType.Identity,
                                    bias=rowfb[:, 4 * c + qb:4 * c + qb + 1],
                                )
                    else:
                        nc.scalar.copy(out=ot[:, u * 1024:(u + 1) * 1024],
                                       in_=pm2[:])
                # Col+Const+bias via broadcast-AP adds: Pool takes the first
                # pair (it is idle in the store phase), DVE the other three
                otv0 = vap(ot, 0, [[M, 8], [1, M]])
                cfb0 = vap(colf, 0, [[0, 8], [1, M]])
                nc.gpsimd.tensor_tensor(out=otv0, in0=otv0, in1=cfb0, op=ADD)
                otv = vap(ot, 1024, [[M, 24], [1, M]])
                cfb = vap(colf, 0, [[0, 24], [1, M]])
                nc.vector.tensor_tensor(out=otv, in0=otv, in1=cfb, op=ADD)
                # diagonal: 32 sparse adds in one strided op
                dview = vap(ot, 32 * g, [[516, 8], [129, 4]])
                dsrc = vap(diaf, 32 * g, [[4, 8], [1, 4]])
                nc.vector.tensor_tensor(out=dview, in0=dview, in1=dsrc, op=ADD)
                nc.gpsimd.dma_start(out=out_d[:, g * GW:(g + 1) * GW], in_=ot[:])

    nc.compile()
    return nc


def _get_nc():
    if "nc" not in _cache:
        _cache["nc"] = _build_program()
    return _cache["nc"]


def _host_prep(coefs, bias, diag_bias):
    import ml_dtypes

    m = float(M)
    C = np.asarray(coefs, dtype=np.float32)

    def bd(b, scale=1.0):
        return C[:, :, b] * np.float32(scale)

    # [15, D, S] pre-scaled coef blocks -> transpose to compact [D, 15, S]
    wmats = np.stack([
        bd(9),              # W_X
        bd(10),             # W_XT
        bd(5, 1 / m),       # W_ROW_CS
        bd(6, 1 / m),       # W_ROW_RS
        bd(11),             # W_ROW_DG
        bd(7, 1 / m),       # W_COL_CS
        bd(8, 1 / m),       # W_COL_RS
        bd(12),             # W_COL_DG
        bd(0),              # W_DIA_DG
        bd(2, 1 / m),       # W_DIA_RS
        bd(3, 1 / m),       # W_DIA_CS
        bd(1, 1 / m),       # W_SD_SD
        bd(4, 1 / (m * m)),  # W_SD_TOT
        bd(13, 1 / m),      # W_SC_SD
        bd(14, 1 / (m * m)),  # W_SC_TOT
    ]).astype(np.float32)
    wmats = np.ascontiguousarray(
        wmats.transpose(1, 0, 2).astype(ml_dtypes.bfloat16))
    bcols = np.stack([
        np.tile(np.asarray(bias, np.float32).reshape(S), NPC),
        np.tile(np.asarray(diag_bias, np.float32).reshape(S), NPC),
    ], axis=1).astype(np.float32)
    return wmats, np.ascontiguousarray(bcols)


def _in_maps(inputs, coefs, bias, diag_bias):
    import ml_dtypes

    x = np.asarray(inputs, np.float32).astype(ml_dtypes.bfloat16)
    wmats, bcols = _host_prep(coefs, bias, diag_bias)
    adiag = np.zeros((P, M), dtype=ml_dtypes.bfloat16)
    for k in range(M):
        adiag[k, (M - 1) - k] = 1.0
    maps = []
    for i in range(NCORES):
        xr = x[i * NPC:(i + 1) * NPC].reshape(P, FREE)
        maps.append({"xr": np.ascontiguousarray(xr), "wmats": wmats,
                     "bcols": bcols, "adiag": adiag})
    return maps


def run(inputs, coefs, bias, diag_bias, **spmd_kwargs):
    """Run on the 8 NeuronCores; returns (output, BassKernelResults)."""
    from concourse.bass_utils import run_bass_kernel_spmd

    nc = _get_nc()
    maps = _in_maps(inputs, coefs, bias, diag_bias)
    res = run_bass_kernel_spmd(nc, maps, list(range(NCORES)), **spmd_kwargs)
    out = np.concatenate(
        [np.asarray(r["outr"]).astype(np.float32).reshape(NPC, S, M, M)
         for r in res.results], axis=0
    )
    return np.ascontiguousarray(out), res


def kernel(inputs, coefs, bias, diag_bias):
    out, _ = run(inputs, coefs, bias, diag_bias)
    return out
